# revision 20
# baseline (speedup 1.0000x reference)
"""Trainium2 Bass kernel for CustomMultiHeadAttention.

Problem: x[2,2048,1024], 16 heads, Dh=64. y = MHA(x) with Q/K/V/O projections.

Sharding (8 cores, no collectives):
  core c -> batch b = c//4, head-quarter hq = c%4 (4 heads, 256 model cols).
  Each core computes Q,K,V projections for its 4 heads over the FULL batch
  sequence, attention for those heads, and a PARTIAL o_proj (its 256 rows of
  Wo). The host sums the 4 partials per batch and adds bo + bv@Wo (the
  "all-reduce after o_proj" done at unshard time; bv is folded into the final
  bias since softmax rows sum to 1).

On-chip dataflow (everything stays transposed, d-on-partitions):
  xT   [1024, 2048]  (host-pretransposed, bf16)
  K^T  = lhsT=Wk_s[d_in,256], rhs=xT -> [256, 2048] bf16 (+bk per-partition)
  V    = lhsT=xT tile, rhs=Wv_s -> [2048, 256] bf16 (ones column appended)
  Q^T  like K^T, emitted per query tile -> [256, 2048] bf16 (+bq)
  S^T  per head = lhsT=K^T[64,k128], rhs=Q^T[64,q512]; head pairs row-packed
       (h2=0 on partitions 0..63, h2=1 on 64..127) so the pair runs
       concurrently in the PE array.
  P^T  = exp(S^T * 0.125) on ACT, batched 1024-wide over two key tiles
       (no max subtraction; |scores| <= ~9).
  O'^T = lhsT=[V|ones], rhs=P^T accumulated over key tiles -> O^T + sumexp row
  O^T  = O'^T * (1/sumexp broadcast across partitions via GpSimd)
  y    = lhsT=O^T[256,q128], rhs=Wo_s[256,1024] -> partial [2048, 1024] f32
"""

import numpy as np
import ml_dtypes

import concourse.bass as bass
import concourse.mybir as mybir
import concourse.tile as tile
from concourse import bacc
from concourse.bass_utils import run_bass_kernel_spmd

P = 128
S = 2048
D = 1024
H = 16
DH = 64
HPC = 4          # heads per core
HD = HPC * DH    # 256 model cols per core
KO = D // P      # 8 contraction subtiles for the projections
KT_N = S // P    # 16 key tiles
QT = 512         # query tile (matmul free dim)
QT_N = S // QT   # 4 query tiles
N_CORES = 8

BF16 = mybir.dt.bfloat16
F32 = mybir.dt.float32
EXP = mybir.ActivationFunctionType.Exp
MUL = mybir.AluOpType.mult

_CACHE = {}


def _build_program():
    nc = bacc.Bacc(
        "TRN2",
        target_bir_lowering=False,
        debug=False,
        enable_asserts=False,
        num_devices=N_CORES,
    )
    xT = nc.dram_tensor("xT", [D, S], BF16, kind="ExternalInput").ap()
    wq = nc.dram_tensor("wq", [D, HD], BF16, kind="ExternalInput").ap()
    wk = nc.dram_tensor("wk", [D, HD], BF16, kind="ExternalInput").ap()
    wv = nc.dram_tensor("wv", [D, HD], BF16, kind="ExternalInput").ap()
    wo = nc.dram_tensor("wo", [HD, D], BF16, kind="ExternalInput").ap()
    bq = nc.dram_tensor("bq", [HD], F32, kind="ExternalInput").ap()
    bk = nc.dram_tensor("bk", [HD], F32, kind="ExternalInput").ap()
    y = nc.dram_tensor("y", [S, D], F32, kind="ExternalOutput").ap()

    with tile.TileContext(nc) as tc:
        _body(tc, y, xT, wq, wk, wv, wo, bq, bk)
    nc.compile()
    return nc


def _body(tc, y, xT, wq, wk, wv, wo, bq, bk):
    nc = tc.nc
    with (
        tc.tile_pool(name="const", bufs=1) as const,
        tc.tile_pool(name="big", bufs=1) as big,
        tc.tile_pool(name="work", bufs=3) as work,
        tc.tile_pool(name="exps", bufs=8) as exps,
        tc.tile_pool(name="yst", bufs=3) as yst,
        tc.tile_pool(name="psw", bufs=2, space="PSUM") as psw,  # wide [P,1024]
        tc.tile_pool(name="psa", bufs=4, space="PSUM") as psa,  # [P,512] tiles
    ):
        # ---- constant / persistent tiles ----
        # wk first: the warmup matmuls and the K projection depend on it
        wk_sb = const.tile([P, KO, HD], BF16, tag="wk")
        nc.sync.dma_start(wk_sb[:], wk.rearrange("(ko p) m -> p ko m", p=P))
        bk_sb = const.tile([P, HD // P], F32, tag="bk")
        nc.sync.dma_start(bk_sb[:], bk.rearrange("(o p) -> p o", p=P))

        # xT loaded in 4 column chunks so the K projection can start early
        xT_sb = big.tile([P, KO, S], BF16, tag="xT")
        xT_r = xT.rearrange("(ko p) s -> p ko s", p=P)
        for c in range(QT_N):
            cs = slice(c * QT, (c + 1) * QT)
            nc.sync.dma_start(xT_sb[:, :, cs], xT_r[:, :, cs])

        wv_sb = const.tile([P, KO, HD], BF16, tag="wv")
        nc.sync.dma_start(wv_sb[:], wv.rearrange("(ko p) m -> p ko m", p=P))
        wq_sb = const.tile([P, KO, HD], BF16, tag="wq")
        nc.sync.dma_start(wq_sb[:], wq.rearrange("(ko p) m -> p ko m", p=P))
        wo_sb = const.tile([P, HD // P, D], BF16, tag="wo")
        nc.sync.dma_start(wo_sb[:], wo.rearrange("(ks p) n -> p ks n", p=P))
        bq_sb = const.tile([P, HD // P], F32, tag="bq")
        nc.sync.dma_start(bq_sb[:], bq.rearrange("(o p) -> p o", p=P))

        # V with a trailing ones column: cols 0..63 = V, col 64 = 1, so the
        # AV matmul produces O^T rows 0..63 plus the sumexp row 64.
        v_sb = big.tile([P, KT_N, HPC, DH + 1], BF16, tag="v")
        nc.vector.memset(v_sb[:, :, :, DH : DH + 1], 1.0)

        qT_sb = big.tile([P, HD // P, S], BF16, tag="qT")
        kT_sb = big.tile([P, HD // P, S], BF16, tag="kT")
        oT_sb = big.tile([P, HD // P, S], BF16, tag="oT")

        # selector matrix for the 1/sumexp partition-broadcast matmul:
        # lhsT = sel0[64:128] (first row of that slice = partition 64 = ones),
        # rhs = rb[64:128] (row 64 = 1/sumexp bf16, rows 65+ stay zero), so
        # out = pb[0:64] = 1/sumexp broadcast to 64 partitions.
        sel0 = const.tile([P, DH], BF16, tag="sel0")
        nc.vector.memset(sel0[:], 0.0)
        nc.vector.memset(sel0[DH : DH + 1, :], 1.0)
        rbs = [const.tile([P, QT], BF16, tag=f"rb{i}", name=f"rb{i}") for i in range(2)]
        for t in rbs:
            nc.vector.memset(t[:], 0.0)

        # ---- PE warmup: dummy matmuls on a zeroed tile (no DMA dependency)
        # so the tensor engine is at full clock when the projections start ----
        wu = const.tile([P, QT], BF16, tag="wu")
        nc.vector.memset(wu[:], 0.0)
        pwu = psa.tile([P, QT], F32, tag="psa", name="pwu")
        for i in range(12):
            nc.tensor.matmul(
                pwu[:], lhsT=wu[:, 0:P], rhs=wu[:], start=True, stop=True
            )

        def k_proj(mt):
            for ntp in range(2):  # pairs of 512-wide column tiles
                pk = psw.tile([P, 2, QT], F32, tag="psw", name="pk")
                for j in range(2):
                    nt = 2 * ntp + j
                    for ko in range(KO):
                        nc.tensor.matmul(
                            pk[:, j, :],
                            lhsT=wk_sb[:, ko, mt * P : (mt + 1) * P],
                            rhs=xT_sb[:, ko, nt * QT : (nt + 1) * QT],
                            start=(ko == 0),
                            stop=(ko == KO - 1),
                        )
                nc.vector.tensor_scalar_add(
                    kT_sb[:, mt, ntp * 2 * QT : (ntp + 1) * 2 * QT],
                    pk.rearrange("p a b -> p (a b)"),
                    bk_sb[:, mt : mt + 1],
                )

        def v_proj(half):
            for stp in range(half * (KT_N // 4), (half + 1) * (KT_N // 4)):
                pv = psw.tile([P, 2, QT], F32, tag="psw", name="pv")
                for j in range(2):
                    st = 2 * stp + j
                    for ko in range(KO):
                        nc.tensor.matmul(
                            pv[:, j, :HD],
                            lhsT=xT_sb[:, ko, st * P : (st + 1) * P],
                            rhs=wv_sb[:, ko, :],
                            start=(ko == 0),
                            stop=(ko == KO - 1),
                        )
                for j in range(2):
                    st = 2 * stp + j
                    nc.vector.tensor_copy(
                        out=v_sb[:, st, :, 0:DH],
                        in_=pv[:, j, :HD].rearrange("p (h c) -> p h c", h=HPC),
                    )

        # ---- per query tile: Q^T projection, attention, partial o_proj ----
        def q_proj(qt):
            qs = slice(qt * QT, (qt + 1) * QT)
            pq = psw.tile([P, 2, QT], F32, tag="psw", name="pq")
            for mt in range(HD // P):
                for ko in range(KO):
                    nc.tensor.matmul(
                        pq[:, mt, :],
                        lhsT=wq_sb[:, ko, mt * P : (mt + 1) * P],
                        rhs=xT_sb[:, ko, qs],
                        start=(ko == 0),
                        stop=(ko == KO - 1),
                    )
            for mt in range(HD // P):
                nc.vector.tensor_scalar_add(
                    qT_sb[:, mt, qs], pq[:, mt, :], bq_sb[:, mt : mt + 1]
                )

        # Emission order biases the scheduler's priorities. Pipeline shape:
        # QK+exp of pair (qt,hp) is emitted BEFORE the AV of the previous
        # pair, so ACT (the attention-phase bottleneck) never waits behind
        # lower-priority AV matmuls at pair boundaries.
        def qk_phase(qt, hp):
            qs = slice(qt * QT, (qt + 1) * QT)
            # exp tiles split into key-tile halves so downstream AV frees
            # pool slots progressively (finer pipeline granularity)
            exp_t = [
                [
                    exps.tile(
                        [P, KT_N // 2, QT], BF16, tag="exps", name=f"exp_{h2i}{ha}"
                    )
                    for ha in range(2)
                ]
                for h2i in range(2)
            ]
            for step in range(KT_N // 2):
                kt = 2 * step
                half, lkt = step // 4, kt % (KT_N // 2)
                for h2 in range(2):
                    pr = slice(h2 * DH, (h2 + 1) * DH)
                    pqk = psw.tile([P, 2, QT], F32, tag="psw", name="pqk")
                    for j in range(2):
                        nc.tensor.matmul(
                            pqk[:, j, :],
                            lhsT=kT_sb[pr, hp, (kt + j) * P : (kt + j + 1) * P],
                            rhs=qT_sb[pr, hp, qs],
                            start=True,
                            stop=True,
                        )
                    nc.scalar.activation(
                        exp_t[h2][half][:, lkt : lkt + 2, :], pqk[:], EXP, scale=0.125
                    )
            return exp_t

        def av_phase(qt, hp, exp_t):
            qs = slice(qt * QT, (qt + 1) * QT)
            for h2 in range(2):
                h = 2 * hp + h2
                po = psa.tile([P, QT], F32, tag="psa", name="po")
                for kt in range(KT_N):
                    nc.tensor.matmul(
                        po[0 : DH + 1, :],  # rows 0..63 O^T, row 64 sumexp
                        lhsT=v_sb[:, kt, h, :],
                        rhs=exp_t[h2][kt // (KT_N // 2)][:, kt % (KT_N // 2), :],
                        start=(kt == 0),
                        stop=(kt == KT_N - 1),
                    )
                # copy out of PSUM right away so the accumulator frees
                pon = work.tile([DH + 1, QT], F32, tag="pon", name="pon")
                nc.vector.tensor_copy(out=pon[:], in_=po[0 : DH + 1, :])
                rb = rbs[(2 * qt + hp) % 2]
                with nc.allow_low_precision(
                    reason="bf16 1/sumexp costs ~0.2% rel err, within budget"
                ):
                    nc.vector.reciprocal(rb[DH : DH + 1, :], pon[DH : DH + 1, :])
                pb = psa.tile([P, QT], F32, tag="psa", name="pb")
                nc.tensor.matmul(
                    pb[0:DH, :],
                    lhsT=sel0[DH:P, :],
                    rhs=rb[DH:P, :],
                    start=True,
                    stop=True,
                )
                if h2 == 0:
                    # even head: O^T rows live on partitions 0..63 of oT
                    nc.vector.tensor_tensor(
                        oT_sb[0:DH, hp, qs], pon[0:DH, :], pb[0:DH, :], MUL
                    )
                else:
                    # odd head: rows belong on partitions 64..127; DVE is
                    # lane-locked, so stage in bf16 and lane-shift via DMA
                    sh = work.tile([DH, QT], BF16, tag="shift", name="sh")
                    nc.vector.tensor_tensor(sh[:], pon[0:DH, :], pb[0:DH, :], MUL)
                    nc.sync.dma_start(oT_sb[DH:P, hp, qs], sh[:])

        def o_proj(qt):
            for st in range(QT // P):
                yt = yst.tile([P, D], F32, tag="yt", name="yt")
                rows = slice(qt * QT + st * P, qt * QT + (st + 1) * P)
                for nt2 in range(D // QT):
                    py = psa.tile([P, QT], F32, tag="psa", name="py")
                    for ks in range(HD // P):
                        nc.tensor.matmul(
                            py[:],
                            lhsT=oT_sb[:, ks, rows],
                            rhs=wo_sb[:, ks, nt2 * QT : (nt2 + 1) * QT],
                            start=(ks == 0),
                            stop=(ks == HD // P - 1),
                        )
                    nc.vector.tensor_copy(
                        out=yt[:, nt2 * QT : (nt2 + 1) * QT], in_=py[:]
                    )
                nc.sync.dma_start(y[rows, :], yt[:])

        # software-pipelined emission schedule: each pair's QK+exp is always
        # emitted before the previous pair's AV, so ACT never waits behind
        # lower-priority AV matmuls; half-split exp tiles let the prologue
        # flow (AV frees slots in 8-key-tile chunks)
        k_proj(0)
        q_proj(0)
        e = {(0, 0): qk_phase(0, 0)}
        k_proj(1)
        q_proj(1)
        e[(0, 1)] = qk_phase(0, 1)
        v_proj(0)
        v_proj(1)
        for qt in range(1, QT_N):
            if qt + 1 < QT_N:
                q_proj(qt + 1)
            e[(qt, 0)] = qk_phase(qt, 0)
            av_phase(qt - 1, 0, e.pop((qt - 1, 0)))
            av_phase(qt - 1, 1, e.pop((qt - 1, 1)))
            o_proj(qt - 1)
            e[(qt, 1)] = qk_phase(qt, 1)
        last = QT_N - 1
        av_phase(last, 0, e.pop((last, 0)))
        av_phase(last, 1, e.pop((last, 1)))
        o_proj(last)


def _prep_inputs(x, Wq, bq, Wk, bk, Wv, bv, Wo, bo):
    bf = ml_dtypes.bfloat16
    x = np.asarray(x, np.float32)
    in_maps = []
    for c in range(N_CORES):
        b, hq = c // 4, c % 4
        cs = slice(hq * HD, (hq + 1) * HD)
        in_maps.append(
            {
                "xT": np.ascontiguousarray(x[b].T).astype(bf),
                "wq": np.ascontiguousarray(np.asarray(Wq, np.float32)[:, cs]).astype(bf),
                "wk": np.ascontiguousarray(np.asarray(Wk, np.float32)[:, cs]).astype(bf),
                "wv": np.ascontiguousarray(np.asarray(Wv, np.float32)[:, cs]).astype(bf),
                "wo": np.ascontiguousarray(np.asarray(Wo, np.float32)[cs, :]).astype(bf),
                "bq": np.ascontiguousarray(np.asarray(bq, np.float32)[cs]),
                "bk": np.ascontiguousarray(np.asarray(bk, np.float32)[cs]),
            }
        )
    return in_maps


def get_program():
    if "nc" not in _CACHE:
        _CACHE["nc"] = _build_program()
    return _CACHE["nc"]


def run(inputs, **kw):
    nc = get_program()
    in_maps = _prep_inputs(**inputs)
    res = run_bass_kernel_spmd(nc, in_maps, core_ids=list(range(N_CORES)), **kw)
    # final bias: bo + bv @ Wo (bv folds out of attention since softmax rows
    # sum to 1), computed in fp32 on host
    bias = np.asarray(inputs["bo"], np.float32) + np.asarray(
        inputs["bv"], np.float32
    ) @ np.asarray(inputs["Wo"], np.float32)
    out = np.empty((2, S, D), np.float32)
    for b in range(2):
        acc = res.results[4 * b]["y"].astype(np.float32).copy()
        for i in range(1, 4):
            acc += res.results[4 * b + i]["y"]
        out[b] = acc + bias
    return out, res


def kernel(**inputs):
    out, _ = run(inputs)
    return out


# revision 24
# speedup vs baseline: 1.0283x; 1.0283x over previous
"""Trainium2 Bass kernel for CustomMultiHeadAttention.

Problem: x[2,2048,1024], 16 heads, Dh=64. y = MHA(x) with Q/K/V/O projections.

Sharding (8 cores, no collectives):
  core c -> batch b = c//4, head-quarter hq = c%4 (4 heads, 256 model cols).
  Each core computes Q,K,V projections for its 4 heads over the FULL batch
  sequence, attention for those heads, and a PARTIAL o_proj (its 256 rows of
  Wo). The host sums the 4 partials per batch and adds bo + bv@Wo (the
  "all-reduce after o_proj" done at unshard time; bv is folded into the final
  bias since softmax rows sum to 1).

On-chip dataflow (everything stays transposed, d-on-partitions):
  xT   [1024, 2048]  (host-pretransposed, bf16)
  K^T  = lhsT=Wk_s[d_in,256], rhs=xT -> [256, 2048] bf16 (+bk per-partition)
  V    = lhsT=xT tile, rhs=Wv_s -> [2048, 256] bf16 (ones column appended)
  Q^T  like K^T, emitted per query tile -> [256, 2048] bf16 (+bq)
  S^T  per head = lhsT=K^T[64,k128], rhs=Q^T[64,q512]; head pairs row-packed
       (h2=0 on partitions 0..63, h2=1 on 64..127) so the pair runs
       concurrently in the PE array.
  P^T  = exp(S^T * 0.125) on ACT, batched 1024-wide over two key tiles
       (no max subtraction; |scores| <= ~9).
  O'^T = lhsT=[V|ones], rhs=P^T accumulated over key tiles -> O^T + sumexp row
  O^T  = O'^T * (1/sumexp broadcast across partitions via GpSimd)
  y    = lhsT=O^T[256,q128], rhs=Wo_s[256,1024] -> partial [2048, 1024] f32
"""

import numpy as np
import ml_dtypes

import concourse.bass as bass
import concourse.mybir as mybir
import concourse.tile as tile
from concourse import bacc
from concourse.bass_utils import run_bass_kernel_spmd

P = 128
S = 2048
D = 1024
H = 16
DH = 64
HPC = 4          # heads per core
HD = HPC * DH    # 256 model cols per core
KO = D // P      # 8 contraction subtiles for the projections
KT_N = S // P    # 16 key tiles
QT = 512         # query tile (matmul free dim)
QT_N = S // QT   # 4 query tiles
N_CORES = 8

BF16 = mybir.dt.bfloat16
F32 = mybir.dt.float32
EXP = mybir.ActivationFunctionType.Exp
MUL = mybir.AluOpType.mult

_CACHE = {}


def _build_program():
    nc = bacc.Bacc(
        "TRN2",
        target_bir_lowering=False,
        debug=False,
        enable_asserts=False,
        num_devices=N_CORES,
    )
    xT = nc.dram_tensor("xT", [D, S], BF16, kind="ExternalInput").ap()
    wq = nc.dram_tensor("wq", [D, HD], BF16, kind="ExternalInput").ap()
    wk = nc.dram_tensor("wk", [D, HD], BF16, kind="ExternalInput").ap()
    wv = nc.dram_tensor("wv", [D, HD], BF16, kind="ExternalInput").ap()
    wo = nc.dram_tensor("wo", [HD, D], BF16, kind="ExternalInput").ap()
    bq = nc.dram_tensor("bq", [HD], F32, kind="ExternalInput").ap()
    bk = nc.dram_tensor("bk", [HD], F32, kind="ExternalInput").ap()
    y = nc.dram_tensor("y", [S, D], F32, kind="ExternalOutput").ap()

    with tile.TileContext(nc) as tc:
        _body(tc, y, xT, wq, wk, wv, wo, bq, bk)
    nc.compile()
    return nc


def _body(tc, y, xT, wq, wk, wv, wo, bq, bk):
    nc = tc.nc
    with (
        tc.tile_pool(name="const", bufs=1) as const,
        tc.tile_pool(name="big", bufs=1) as big,
        tc.tile_pool(name="work", bufs=3) as work,
        tc.tile_pool(name="exps", bufs=8) as exps,
        tc.tile_pool(name="yst", bufs=3) as yst,
        tc.tile_pool(name="psw", bufs=2, space="PSUM") as psw,  # wide [P,1024]
        tc.tile_pool(name="psa", bufs=4, space="PSUM") as psa,  # [P,512] tiles
    ):
        # ---- constant / persistent tiles ----
        # wk first: the warmup matmuls and the K projection depend on it
        wk_sb = const.tile([P, KO, HD], BF16, tag="wk")
        nc.sync.dma_start(wk_sb[:], wk.rearrange("(ko p) m -> p ko m", p=P))
        bk_sb = const.tile([P, HD // P], F32, tag="bk")
        nc.sync.dma_start(bk_sb[:], bk.rearrange("(o p) -> p o", p=P))

        # xT loaded in 4 column chunks so the K projection can start early
        xT_sb = big.tile([P, KO, S], BF16, tag="xT")
        xT_r = xT.rearrange("(ko p) s -> p ko s", p=P)
        for c in range(QT_N):
            cs = slice(c * QT, (c + 1) * QT)
            nc.sync.dma_start(xT_sb[:, :, cs], xT_r[:, :, cs])

        wv_sb = const.tile([P, KO, HD], BF16, tag="wv")
        nc.sync.dma_start(wv_sb[:], wv.rearrange("(ko p) m -> p ko m", p=P))
        wq_sb = const.tile([P, KO, HD], BF16, tag="wq")
        nc.sync.dma_start(wq_sb[:], wq.rearrange("(ko p) m -> p ko m", p=P))
        wo_sb = const.tile([P, HD // P, D], BF16, tag="wo")
        nc.sync.dma_start(wo_sb[:], wo.rearrange("(ks p) n -> p ks n", p=P))
        bq_sb = const.tile([P, HD // P], F32, tag="bq")
        nc.sync.dma_start(bq_sb[:], bq.rearrange("(o p) -> p o", p=P))

        # V with a trailing ones column: cols 0..63 = V, col 64 = 1, so the
        # AV matmul produces O^T rows 0..63 plus the sumexp row 64.
        v_sb = big.tile([P, KT_N, HPC, DH + 1], BF16, tag="v")
        nc.vector.memset(v_sb[:, :, :, DH : DH + 1], 1.0)

        qT_sb = big.tile([P, HD // P, S], BF16, tag="qT")
        kT_sb = big.tile([P, HD // P, S], BF16, tag="kT")
        oT_sb = big.tile([P, HD // P, S], BF16, tag="oT")

        # selector matrix for the 1/sumexp partition-broadcast matmul:
        # lhsT = sel0[64:128] (first row of that slice = partition 64 = ones),
        # rhs = rb[64:128] (row 64 = 1/sumexp bf16, rows 65+ stay zero), so
        # out = pb[0:64] = 1/sumexp broadcast to 64 partitions.
        sel0 = const.tile([P, DH], BF16, tag="sel0")
        nc.vector.memset(sel0[:], 0.0)
        nc.vector.memset(sel0[DH : DH + 1, :], 1.0)
        rbs = [const.tile([P, QT], BF16, tag=f"rb{i}", name=f"rb{i}") for i in range(2)]
        for t in rbs:
            nc.vector.memset(t[:], 0.0)

        # ---- PE warmup: dummy matmuls on a zeroed tile (no DMA dependency)
        # so the tensor engine is at full clock when the projections start ----
        wu = const.tile([P, QT], BF16, tag="wu")
        nc.vector.memset(wu[:], 0.0)
        pwu = psa.tile([P, QT], F32, tag="psa", name="pwu")
        for i in range(12):
            nc.tensor.matmul(
                pwu[:], lhsT=wu[:, 0:P], rhs=wu[:], start=True, stop=True
            )

        def k_proj(mt, ntps=(0, 1)):
            for ntp in ntps:  # pairs of 512-wide column tiles
                pk = psw.tile([P, 2, QT], F32, tag="psw", name="pk")
                for j in range(2):
                    nt = 2 * ntp + j
                    for ko in range(KO):
                        nc.tensor.matmul(
                            pk[:, j, :],
                            lhsT=wk_sb[:, ko, mt * P : (mt + 1) * P],
                            rhs=xT_sb[:, ko, nt * QT : (nt + 1) * QT],
                            start=(ko == 0),
                            stop=(ko == KO - 1),
                        )
                nc.vector.tensor_scalar_add(
                    kT_sb[:, mt, ntp * 2 * QT : (ntp + 1) * 2 * QT],
                    pk.rearrange("p a b -> p (a b)"),
                    bk_sb[:, mt : mt + 1],
                )

        def v_proj(half):
            for stp in range(half * (KT_N // 4), (half + 1) * (KT_N // 4)):
                pv = psw.tile([P, 2, QT], F32, tag="psw", name="pv")
                for j in range(2):
                    st = 2 * stp + j
                    for ko in range(KO):
                        nc.tensor.matmul(
                            pv[:, j, :HD],
                            lhsT=xT_sb[:, ko, st * P : (st + 1) * P],
                            rhs=wv_sb[:, ko, :],
                            start=(ko == 0),
                            stop=(ko == KO - 1),
                        )
                for j in range(2):
                    st = 2 * stp + j
                    nc.vector.tensor_copy(
                        out=v_sb[:, st, :, 0:DH],
                        in_=pv[:, j, :HD].rearrange("p (h c) -> p h c", h=HPC),
                    )

        # ---- per query tile: Q^T projection, attention, partial o_proj ----
        def q_proj(qt):
            qs = slice(qt * QT, (qt + 1) * QT)
            pq = psw.tile([P, 2, QT], F32, tag="psw", name="pq")
            for mt in range(HD // P):
                for ko in range(KO):
                    nc.tensor.matmul(
                        pq[:, mt, :],
                        lhsT=wq_sb[:, ko, mt * P : (mt + 1) * P],
                        rhs=xT_sb[:, ko, qs],
                        start=(ko == 0),
                        stop=(ko == KO - 1),
                    )
            for mt in range(HD // P):
                nc.vector.tensor_scalar_add(
                    qT_sb[:, mt, qs], pq[:, mt, :], bq_sb[:, mt : mt + 1]
                )

        # Emission order biases the scheduler's priorities. Pipeline shape:
        # QK+exp of pair (qt,hp) is emitted BEFORE the AV of the previous
        # pair, so ACT (the attention-phase bottleneck) never waits behind
        # lower-priority AV matmuls at pair boundaries.
        def qk_phase(qt, hp):
            qs = slice(qt * QT, (qt + 1) * QT)
            # exp tiles split into key-tile halves so downstream AV frees
            # pool slots progressively (finer pipeline granularity)
            exp_t = [
                [
                    exps.tile(
                        [P, KT_N // 2, QT], BF16, tag="exps", name=f"exp_{h2i}{ha}"
                    )
                    for ha in range(2)
                ]
                for h2i in range(2)
            ]
            for step in range(KT_N // 2):
                kt = 2 * step
                half, lkt = step // 4, kt % (KT_N // 2)
                for h2 in range(2):
                    pr = slice(h2 * DH, (h2 + 1) * DH)
                    pqk = psw.tile([P, 2, QT], F32, tag="psw", name="pqk")
                    for j in range(2):
                        nc.tensor.matmul(
                            pqk[:, j, :],
                            lhsT=kT_sb[pr, hp, (kt + j) * P : (kt + j + 1) * P],
                            rhs=qT_sb[pr, hp, qs],
                            start=True,
                            stop=True,
                        )
                    nc.scalar.activation(
                        exp_t[h2][half][:, lkt : lkt + 2, :], pqk[:], EXP, scale=0.125
                    )
            return exp_t

        def av_phase(qt, hp, exp_t):
            qs = slice(qt * QT, (qt + 1) * QT)
            for h2 in range(2):
                h = 2 * hp + h2
                po = psa.tile([P, QT], F32, tag="psa", name="po")
                for kt in range(KT_N):
                    nc.tensor.matmul(
                        po[0 : DH + 1, :],  # rows 0..63 O^T, row 64 sumexp
                        lhsT=v_sb[:, kt, h, :],
                        rhs=exp_t[h2][kt // (KT_N // 2)][:, kt % (KT_N // 2), :],
                        start=(kt == 0),
                        stop=(kt == KT_N - 1),
                    )
                # reciprocal straight from PSUM first (it gates the PE
                # broadcast matmul), then stage O^T rows out of PSUM
                rb = rbs[(2 * qt + hp) % 2]
                with nc.allow_low_precision(
                    reason="bf16 1/sumexp costs ~0.2% rel err, within budget"
                ):
                    nc.vector.reciprocal(rb[DH : DH + 1, :], po[DH : DH + 1, :])
                pon = work.tile([DH, QT], F32, tag="pon", name="pon")
                nc.vector.tensor_copy(out=pon[:], in_=po[0:DH, :])
                pb = psa.tile([P, QT], F32, tag="psa", name="pb")
                nc.tensor.matmul(
                    pb[0:DH, :],
                    lhsT=sel0[DH:P, :],
                    rhs=rb[DH:P, :],
                    start=True,
                    stop=True,
                )
                if h2 == 0:
                    # even head: O^T rows live on partitions 0..63 of oT
                    nc.vector.tensor_tensor(
                        oT_sb[0:DH, hp, qs], pon[:], pb[0:DH, :], MUL
                    )
                else:
                    # odd head: rows belong on partitions 64..127; DVE is
                    # lane-locked, so stage in bf16 and lane-shift via DMA
                    sh = work.tile([DH, QT], BF16, tag="shift", name="sh")
                    nc.vector.tensor_tensor(sh[:], pon[:], pb[0:DH, :], MUL)
                    nc.sync.dma_start(oT_sb[DH:P, hp, qs], sh[:])

        def o_proj(qt):
            for st in range(QT // P):
                yt = yst.tile([P, D], F32, tag="yt", name="yt")
                rows = slice(qt * QT + st * P, qt * QT + (st + 1) * P)
                for nt2 in range(D // QT):
                    py = psa.tile([P, QT], F32, tag="psa", name="py")
                    for ks in range(HD // P):
                        nc.tensor.matmul(
                            py[:],
                            lhsT=oT_sb[:, ks, rows],
                            rhs=wo_sb[:, ks, nt2 * QT : (nt2 + 1) * QT],
                            start=(ks == 0),
                            stop=(ks == HD // P - 1),
                        )
                    nc.vector.tensor_copy(
                        out=yt[:, nt2 * QT : (nt2 + 1) * QT], in_=py[:]
                    )
                    nc.sync.dma_start(
                        y[rows, nt2 * QT : (nt2 + 1) * QT],
                        yt[:, nt2 * QT : (nt2 + 1) * QT],
                    )

        # software-pipelined emission schedule: each pair's QK+exp is always
        # emitted before the previous pair's AV, so ACT never waits behind
        # lower-priority AV matmuls; half-split exp tiles let the prologue
        # flow (AV frees slots in 8-key-tile chunks)
        k_proj(0)
        q_proj(0)
        e = {(0, 0): qk_phase(0, 0)}
        k_proj(1)
        q_proj(1)
        e[(0, 1)] = qk_phase(0, 1)
        v_proj(0)
        v_proj(1)
        for qt in range(1, QT_N):
            if qt + 1 < QT_N:
                q_proj(qt + 1)
            e[(qt, 0)] = qk_phase(qt, 0)
            av_phase(qt - 1, 0, e.pop((qt - 1, 0)))
            av_phase(qt - 1, 1, e.pop((qt - 1, 1)))
            o_proj(qt - 1)
            e[(qt, 1)] = qk_phase(qt, 1)
        last = QT_N - 1
        av_phase(last, 0, e.pop((last, 0)))
        av_phase(last, 1, e.pop((last, 1)))
        o_proj(last)


def _prep_inputs(x, Wq, bq, Wk, bk, Wv, bv, Wo, bo):
    bf = ml_dtypes.bfloat16
    x = np.asarray(x, np.float32)
    in_maps = []
    for c in range(N_CORES):
        b, hq = c // 4, c % 4
        cs = slice(hq * HD, (hq + 1) * HD)
        in_maps.append(
            {
                "xT": np.ascontiguousarray(x[b].T).astype(bf),
                "wq": np.ascontiguousarray(np.asarray(Wq, np.float32)[:, cs]).astype(bf),
                "wk": np.ascontiguousarray(np.asarray(Wk, np.float32)[:, cs]).astype(bf),
                "wv": np.ascontiguousarray(np.asarray(Wv, np.float32)[:, cs]).astype(bf),
                "wo": np.ascontiguousarray(np.asarray(Wo, np.float32)[cs, :]).astype(bf),
                "bq": np.ascontiguousarray(np.asarray(bq, np.float32)[cs]),
                "bk": np.ascontiguousarray(np.asarray(bk, np.float32)[cs]),
            }
        )
    return in_maps


def get_program():
    if "nc" not in _CACHE:
        _CACHE["nc"] = _build_program()
    return _CACHE["nc"]


def run(inputs, **kw):
    nc = get_program()
    in_maps = _prep_inputs(**inputs)
    res = run_bass_kernel_spmd(nc, in_maps, core_ids=list(range(N_CORES)), **kw)
    # final bias: bo + bv @ Wo (bv folds out of attention since softmax rows
    # sum to 1), computed in fp32 on host
    bias = np.asarray(inputs["bo"], np.float32) + np.asarray(
        inputs["bv"], np.float32
    ) @ np.asarray(inputs["Wo"], np.float32)
    out = np.empty((2, S, D), np.float32)
    for b in range(2):
        acc = res.results[4 * b]["y"].astype(np.float32).copy()
        for i in range(1, 4):
            acc += res.results[4 * b + i]["y"]
        out[b] = acc + bias
    return out, res


def kernel(**inputs):
    out, _ = run(inputs)
    return out


# revision 25
# speedup vs baseline: 1.0393x; 1.0107x over previous
"""Trainium2 Bass kernel for CustomMultiHeadAttention.

Problem: x[2,2048,1024], 16 heads, Dh=64. y = MHA(x) with Q/K/V/O projections.

Sharding (8 cores, no collectives):
  core c -> batch b = c//4, head-quarter hq = c%4 (4 heads, 256 model cols).
  Each core computes Q,K,V projections for its 4 heads over the FULL batch
  sequence, attention for those heads, and a PARTIAL o_proj (its 256 rows of
  Wo). The host sums the 4 partials per batch and adds bo + bv@Wo (the
  "all-reduce after o_proj" done at unshard time; bv is folded into the final
  bias since softmax rows sum to 1).

On-chip dataflow (everything stays transposed, d-on-partitions):
  xT   [1024, 2048]  (host-pretransposed, bf16)
  K^T  = lhsT=Wk_s[d_in,256], rhs=xT -> [256, 2048] bf16 (+bk per-partition)
  V    = lhsT=xT tile, rhs=Wv_s -> [2048, 256] bf16 (ones column appended)
  Q^T  like K^T, emitted per query tile -> [256, 2048] bf16 (+bq)
  S^T  per head = lhsT=K^T[64,k128], rhs=Q^T[64,q512]; head pairs row-packed
       (h2=0 on partitions 0..63, h2=1 on 64..127) so the pair runs
       concurrently in the PE array.
  P^T  = exp(S^T * 0.125) on ACT, batched 1024-wide over two key tiles
       (no max subtraction; |scores| <= ~9).
  O'^T = lhsT=[V|ones], rhs=P^T accumulated over key tiles -> O^T + sumexp row
  O^T  = O'^T * (1/sumexp broadcast across partitions via GpSimd)
  y    = lhsT=O^T[256,q128], rhs=Wo_s[256,1024] -> partial [2048, 1024] f32
"""

import numpy as np
import ml_dtypes

import concourse.bass as bass
import concourse.mybir as mybir
import concourse.tile as tile
from concourse import bacc
from concourse.bass_utils import run_bass_kernel_spmd

P = 128
S = 2048
D = 1024
H = 16
DH = 64
HPC = 4          # heads per core
HD = HPC * DH    # 256 model cols per core
KO = D // P      # 8 contraction subtiles for the projections
KT_N = S // P    # 16 key tiles
QT = 512         # query tile (matmul free dim)
QT_N = S // QT   # 4 query tiles
N_CORES = 8

BF16 = mybir.dt.bfloat16
F32 = mybir.dt.float32
EXP = mybir.ActivationFunctionType.Exp
MUL = mybir.AluOpType.mult

_CACHE = {}


def _build_program():
    nc = bacc.Bacc(
        "TRN2",
        target_bir_lowering=False,
        debug=False,
        enable_asserts=False,
        num_devices=N_CORES,
    )
    xT = nc.dram_tensor("xT", [D, S], BF16, kind="ExternalInput").ap()
    wq = nc.dram_tensor("wq", [D, HD], BF16, kind="ExternalInput").ap()
    wk = nc.dram_tensor("wk", [D, HD], BF16, kind="ExternalInput").ap()
    wv = nc.dram_tensor("wv", [D, HD], BF16, kind="ExternalInput").ap()
    wo = nc.dram_tensor("wo", [HD, D], BF16, kind="ExternalInput").ap()
    bq = nc.dram_tensor("bq", [HD], F32, kind="ExternalInput").ap()
    bk = nc.dram_tensor("bk", [HD], F32, kind="ExternalInput").ap()
    y = nc.dram_tensor("y", [S, D], F32, kind="ExternalOutput").ap()

    with tile.TileContext(nc) as tc:
        _body(tc, y, xT, wq, wk, wv, wo, bq, bk)
    nc.compile()
    return nc


def _body(tc, y, xT, wq, wk, wv, wo, bq, bk):
    nc = tc.nc
    with (
        tc.tile_pool(name="const", bufs=1) as const,
        tc.tile_pool(name="big", bufs=1) as big,
        tc.tile_pool(name="work", bufs=3) as work,
        tc.tile_pool(name="exps", bufs=8) as exps,
        tc.tile_pool(name="yst", bufs=3) as yst,
        tc.tile_pool(name="psw", bufs=2, space="PSUM") as psw,  # wide [P,1024]
        tc.tile_pool(name="psa", bufs=4, space="PSUM") as psa,  # [P,512] tiles
    ):
        # ---- constant / persistent tiles ----
        # wk first: the warmup matmuls and the K projection depend on it
        wk_sb = const.tile([P, KO, HD], BF16, tag="wk")
        nc.sync.dma_start(wk_sb[:], wk.rearrange("(ko p) m -> p ko m", p=P))
        bk_sb = const.tile([P, HD // P], F32, tag="bk")
        nc.sync.dma_start(bk_sb[:], bk.rearrange("(o p) -> p o", p=P))

        # xT loaded in 4 column chunks so the K projection can start early
        xT_sb = big.tile([P, KO, S], BF16, tag="xT")
        xT_r = xT.rearrange("(ko p) s -> p ko s", p=P)
        for c in range(QT_N):
            cs = slice(c * QT, (c + 1) * QT)
            nc.sync.dma_start(xT_sb[:, :, cs], xT_r[:, :, cs])

        wv_sb = const.tile([P, KO, HD], BF16, tag="wv")
        nc.sync.dma_start(wv_sb[:], wv.rearrange("(ko p) m -> p ko m", p=P))
        wq_sb = const.tile([P, KO, HD], BF16, tag="wq")
        nc.sync.dma_start(wq_sb[:], wq.rearrange("(ko p) m -> p ko m", p=P))
        wo_sb = const.tile([P, HD // P, D], BF16, tag="wo")
        nc.sync.dma_start(wo_sb[:], wo.rearrange("(ks p) n -> p ks n", p=P))
        bq_sb = const.tile([P, HD // P], F32, tag="bq")
        nc.sync.dma_start(bq_sb[:], bq.rearrange("(o p) -> p o", p=P))

        # V with a trailing ones column: cols 0..63 = V, col 64 = 1, so the
        # AV matmul produces O^T rows 0..63 plus the sumexp row 64.
        v_sb = big.tile([P, KT_N, HPC, DH + 1], BF16, tag="v")
        nc.vector.memset(v_sb[:, :, :, DH : DH + 1], 1.0)

        qT_sb = big.tile([P, HD // P, S], BF16, tag="qT")
        kT_sb = big.tile([P, HD // P, S], BF16, tag="kT")
        oT_sb = big.tile([P, HD // P, S], BF16, tag="oT")

        # selector matrix for the 1/sumexp partition-broadcast matmul:
        # lhsT = sel0[64:128] (first row of that slice = partition 64 = ones),
        # rhs = rb[64:128] (row 64 = 1/sumexp bf16, rows 65+ stay zero), so
        # out = pb[0:64] = 1/sumexp broadcast to 64 partitions.
        sel0 = const.tile([P, DH], BF16, tag="sel0")
        nc.vector.memset(sel0[:], 0.0)
        nc.vector.memset(sel0[DH : DH + 1, :], 1.0)
        rbs = [const.tile([P, QT], BF16, tag=f"rb{i}", name=f"rb{i}") for i in range(2)]
        for t in rbs:
            nc.vector.memset(t[:], 0.0)

        # ---- PE warmup: dummy matmuls on a zeroed tile (no DMA dependency)
        # so the tensor engine is at full clock when the projections start ----
        wu = const.tile([P, QT], BF16, tag="wu")
        nc.vector.memset(wu[:], 0.0)
        pwu = psa.tile([P, QT], F32, tag="psa", name="pwu")
        for i in range(12):
            nc.tensor.matmul(
                pwu[:], lhsT=wu[:, 0:P], rhs=wu[:], start=True, stop=True
            )

        def k_proj(mt, ntps=(0, 1)):
            for ntp in ntps:  # pairs of 512-wide column tiles
                pk = psw.tile([P, 2, QT], F32, tag="psw", name="pk")
                for j in range(2):
                    nt = 2 * ntp + j
                    for ko in range(KO):
                        nc.tensor.matmul(
                            pk[:, j, :],
                            lhsT=wk_sb[:, ko, mt * P : (mt + 1) * P],
                            rhs=xT_sb[:, ko, nt * QT : (nt + 1) * QT],
                            start=(ko == 0),
                            stop=(ko == KO - 1),
                        )
                nc.vector.tensor_scalar_add(
                    kT_sb[:, mt, ntp * 2 * QT : (ntp + 1) * 2 * QT],
                    pk.rearrange("p a b -> p (a b)"),
                    bk_sb[:, mt : mt + 1],
                )

        def v_proj(half):
            for stp in range(half * (KT_N // 4), (half + 1) * (KT_N // 4)):
                pv = psw.tile([P, 2, QT], F32, tag="psw", name="pv")
                for j in range(2):
                    st = 2 * stp + j
                    for ko in range(KO):
                        nc.tensor.matmul(
                            pv[:, j, :HD],
                            lhsT=xT_sb[:, ko, st * P : (st + 1) * P],
                            rhs=wv_sb[:, ko, :],
                            start=(ko == 0),
                            stop=(ko == KO - 1),
                        )
                for j in range(2):
                    st = 2 * stp + j
                    nc.vector.tensor_copy(
                        out=v_sb[:, st, :, 0:DH],
                        in_=pv[:, j, :HD].rearrange("p (h c) -> p h c", h=HPC),
                    )

        # ---- per query tile: Q^T projection, attention, partial o_proj ----
        def q_proj(qt):
            qs = slice(qt * QT, (qt + 1) * QT)
            pq = psw.tile([P, 2, QT], F32, tag="psw", name="pq")
            for mt in range(HD // P):
                for ko in range(KO):
                    nc.tensor.matmul(
                        pq[:, mt, :],
                        lhsT=wq_sb[:, ko, mt * P : (mt + 1) * P],
                        rhs=xT_sb[:, ko, qs],
                        start=(ko == 0),
                        stop=(ko == KO - 1),
                    )
            for mt in range(HD // P):
                nc.vector.tensor_scalar_add(
                    qT_sb[:, mt, qs], pq[:, mt, :], bq_sb[:, mt : mt + 1]
                )

        # Emission order biases the scheduler's priorities. Pipeline shape:
        # QK+exp of pair (qt,hp) is emitted BEFORE the AV of the previous
        # pair, so ACT (the attention-phase bottleneck) never waits behind
        # lower-priority AV matmuls at pair boundaries.
        def qk_phase(qt, hp):
            qs = slice(qt * QT, (qt + 1) * QT)
            # exp tiles split into key-tile halves so downstream AV frees
            # pool slots progressively (finer pipeline granularity)
            exp_t = [
                [
                    exps.tile(
                        [P, KT_N // 2, QT], BF16, tag="exps", name=f"exp_{h2i}{ha}"
                    )
                    for ha in range(2)
                ]
                for h2i in range(2)
            ]
            for step in range(KT_N // 2):
                kt = 2 * step
                half, lkt = step // 4, kt % (KT_N // 2)
                for h2 in range(2):
                    pr = slice(h2 * DH, (h2 + 1) * DH)
                    pqk = psw.tile([P, 2, QT], F32, tag="psw", name="pqk")
                    for j in range(2):
                        nc.tensor.matmul(
                            pqk[:, j, :],
                            lhsT=kT_sb[pr, hp, (kt + j) * P : (kt + j + 1) * P],
                            rhs=qT_sb[pr, hp, qs],
                            start=True,
                            stop=True,
                        )
                    nc.scalar.activation(
                        exp_t[h2][half][:, lkt : lkt + 2, :], pqk[:], EXP, scale=0.125
                    )
            return exp_t

        def av_phase(qt, hp, exp_t, h2_order=(0, 1)):
            qs = slice(qt * QT, (qt + 1) * QT)
            for h2 in h2_order:
                h = 2 * hp + h2
                po = psa.tile([P, QT], F32, tag="psa", name="po")
                for kt in range(KT_N):
                    nc.tensor.matmul(
                        po[0 : DH + 1, :],  # rows 0..63 O^T, row 64 sumexp
                        lhsT=v_sb[:, kt, h, :],
                        rhs=exp_t[h2][kt // (KT_N // 2)][:, kt % (KT_N // 2), :],
                        start=(kt == 0),
                        stop=(kt == KT_N - 1),
                    )
                # reciprocal straight from PSUM first (it gates the PE
                # broadcast matmul), then stage O^T rows out of PSUM
                rb = rbs[(2 * qt + hp) % 2]
                with nc.allow_low_precision(
                    reason="bf16 1/sumexp costs ~0.2% rel err, within budget"
                ):
                    nc.vector.reciprocal(rb[DH : DH + 1, :], po[DH : DH + 1, :])
                pon = work.tile([DH, QT], F32, tag="pon", name="pon")
                nc.vector.tensor_copy(out=pon[:], in_=po[0:DH, :])
                pb = psa.tile([P, QT], F32, tag="psa", name="pb")
                nc.tensor.matmul(
                    pb[0:DH, :],
                    lhsT=sel0[DH:P, :],
                    rhs=rb[DH:P, :],
                    start=True,
                    stop=True,
                )
                if h2 == 0:
                    # even head: O^T rows live on partitions 0..63 of oT
                    nc.vector.tensor_tensor(
                        oT_sb[0:DH, hp, qs], pon[:], pb[0:DH, :], MUL
                    )
                else:
                    # odd head: rows belong on partitions 64..127; DVE is
                    # lane-locked, so stage in bf16 and lane-shift via DMA
                    sh = work.tile([DH, QT], BF16, tag="shift", name="sh")
                    nc.vector.tensor_tensor(sh[:], pon[:], pb[0:DH, :], MUL)
                    nc.sync.dma_start(oT_sb[DH:P, hp, qs], sh[:])

        def o_proj(qt):
            for st in range(QT // P):
                yt = yst.tile([P, D], F32, tag="yt", name="yt")
                rows = slice(qt * QT + st * P, qt * QT + (st + 1) * P)
                for nt2 in range(D // QT):
                    py = psa.tile([P, QT], F32, tag="psa", name="py")
                    for ks in range(HD // P):
                        nc.tensor.matmul(
                            py[:],
                            lhsT=oT_sb[:, ks, rows],
                            rhs=wo_sb[:, ks, nt2 * QT : (nt2 + 1) * QT],
                            start=(ks == 0),
                            stop=(ks == HD // P - 1),
                        )
                    nc.vector.tensor_copy(
                        out=yt[:, nt2 * QT : (nt2 + 1) * QT], in_=py[:]
                    )
                    nc.sync.dma_start(
                        y[rows, nt2 * QT : (nt2 + 1) * QT],
                        yt[:, nt2 * QT : (nt2 + 1) * QT],
                    )

        # software-pipelined emission schedule: each pair's QK+exp is always
        # emitted before the previous pair's AV, so ACT never waits behind
        # lower-priority AV matmuls; half-split exp tiles let the prologue
        # flow (AV frees slots in 8-key-tile chunks)
        k_proj(0)
        q_proj(0)
        e = {(0, 0): qk_phase(0, 0)}
        k_proj(1)
        q_proj(1)
        e[(0, 1)] = qk_phase(0, 1)
        v_proj(0)
        v_proj(1)
        for qt in range(1, QT_N):
            if qt + 1 < QT_N:
                q_proj(qt + 1)
            e[(qt, 0)] = qk_phase(qt, 0)
            av_phase(qt - 1, 0, e.pop((qt - 1, 0)))
            av_phase(qt - 1, 1, e.pop((qt - 1, 1)))
            o_proj(qt - 1)
            e[(qt, 1)] = qk_phase(qt, 1)
        last = QT_N - 1
        av_phase(last, 0, e.pop((last, 0)))
        # odd head first in the final pair: its lane-shift DMA (the last
        # o_proj dependency) overlaps the even head's AV + normalize
        av_phase(last, 1, e.pop((last, 1)), h2_order=(1, 0))
        o_proj(last)


def _prep_inputs(x, Wq, bq, Wk, bk, Wv, bv, Wo, bo):
    bf = ml_dtypes.bfloat16
    x = np.asarray(x, np.float32)
    in_maps = []
    for c in range(N_CORES):
        b, hq = c // 4, c % 4
        cs = slice(hq * HD, (hq + 1) * HD)
        in_maps.append(
            {
                "xT": np.ascontiguousarray(x[b].T).astype(bf),
                "wq": np.ascontiguousarray(np.asarray(Wq, np.float32)[:, cs]).astype(bf),
                "wk": np.ascontiguousarray(np.asarray(Wk, np.float32)[:, cs]).astype(bf),
                "wv": np.ascontiguousarray(np.asarray(Wv, np.float32)[:, cs]).astype(bf),
                "wo": np.ascontiguousarray(np.asarray(Wo, np.float32)[cs, :]).astype(bf),
                "bq": np.ascontiguousarray(np.asarray(bq, np.float32)[cs]),
                "bk": np.ascontiguousarray(np.asarray(bk, np.float32)[cs]),
            }
        )
    return in_maps


def get_program():
    if "nc" not in _CACHE:
        _CACHE["nc"] = _build_program()
    return _CACHE["nc"]


def run(inputs, **kw):
    nc = get_program()
    in_maps = _prep_inputs(**inputs)
    res = run_bass_kernel_spmd(nc, in_maps, core_ids=list(range(N_CORES)), **kw)
    # final bias: bo + bv @ Wo (bv folds out of attention since softmax rows
    # sum to 1), computed in fp32 on host
    bias = np.asarray(inputs["bo"], np.float32) + np.asarray(
        inputs["bv"], np.float32
    ) @ np.asarray(inputs["Wo"], np.float32)
    out = np.empty((2, S, D), np.float32)
    for b in range(2):
        acc = res.results[4 * b]["y"].astype(np.float32).copy()
        for i in range(1, 4):
            acc += res.results[4 * b + i]["y"]
        out[b] = acc + bias
    return out, res


def kernel(**inputs):
    out, _ = run(inputs)
    return out
